# revision 1
# baseline (speedup 1.0000x reference)
"""NetVLAD Trainium2 kernel (Bass/Tile), data-parallel over batch on 8 cores.

Math (per batch b):
    x_hat = x / ||x||_2(channel)                    (B, D, H*W), D=512, N=1200
    logits = conv_w @ x_hat                         (K, N), K=64
    a = softmax_K(logits)
    vlad[k,d] = sum_n a[k,n] * x_hat[d,n] - (sum_n a[k,n]) * c[k,d]
    vlad = l2norm_rows(vlad); out = l2norm(flatten(vlad))   # == vlad_rows/8

Device-side structure:
  - x stays in natural (D-major) layout for the PE: each 128x128 chunk of x is
    the matmul stationary operand, streamed against [identity | conv_wT-chunk],
    producing BOTH the transposed x (n-major, for the aggregation matmul) and
    the logits in n-major layout (softmax along the free axis).
  - normalization scale is folded into the softmax weights instead of x:
        a'[n,k] = a[n,k] / s[n]   =>   vlad term 1 contracts a' with RAW x.
  - rsqrt is computed as exp(-0.5*ln(.)) so the single ACT table set
    natural_log_exp_and_others covers every ScalarE op in the kernel.
  - asum[k] = sum_n a[k,n] obtained by streaming the norm column s through the
    aggregation weights: sum_n a'[n,k]*s[n].
  - PSUM is four immortal tensors with manual region rotation (8 banks exactly)
    because pool slot re-acquisition joins producer+consumer sems into >1 sync
    wait on a Matmult, which walrus's S3_LW struct cannot encode.  For the same
    reason each batch starts with a tiny "warm" matmul that observes the x DMA
    semaphore so the first real transpose only carries the WAR wait.
"""

import numpy as np

import concourse.bass as bass
import concourse.mybir as mybir
from concourse import bacc
import concourse.tile as tile
from concourse.bass_utils import run_bass_kernel_spmd
from concourse.masks import make_identity
from concourse.tile_rust import add_dep_helper

F32 = mybir.dt.float32
F32R = mybir.dt.float32r
ALU = mybir.AluOpType
ACTF = mybir.ActivationFunctionType

P = 128
BPC = 8            # batches per core
D = 512
N = 1200
K = 64
DCH = D // P       # 4 d-chunks
NCHUNKS = [(j * P, min(P, N - j * P)) for j in range((N + P - 1) // P)]  # 10
NJ = len(NCHUNKS)
LN_EIGHTH = float(np.log(0.125))

# chunk-index splits for balancing PSUM->SBUF copies / sumsq between ACT & DVE
ACT_COPY_CHUNKS = 6   # chunks [0, ACT_COPY_CHUNKS) copied by ScalarE, rest DVE
ACT_SQ_CHUNKS = 5     # chunks [0, ACT_SQ_CHUNKS) squared+accum by ScalarE


def _emit(nc):
    x = nc.dram_tensor("x", (BPC, D, N), F32R, kind="ExternalInput")
    wt = nc.dram_tensor("wt", (D, K), F32R, kind="ExternalInput")
    cent = nc.dram_tensor("cent", (K, D), F32, kind="ExternalInput")
    out = nc.dram_tensor("out", (BPC, K, D), F32, kind="ExternalOutput")

    with tile.TileContext(nc) as tc:
        with (
            tc.tile_pool(name="const", bufs=1) as const,
            tc.tile_pool(name="xnat", bufs=4) as xnat_pool,
            tc.tile_pool(name="xtsb", bufs=2) as xt_pool,
            tc.tile_pool(name="softmax", bufs=2) as sm_pool,
            tc.tile_pool(name="smalls", bufs=2) as smalls,
            tc.tile_pool(name="scratch", bufs=3) as scratch,
            tc.tile_pool(name="epilog", bufs=2) as ep_pool,
            tc.tile_pool(name="psum", bufs=1, space="PSUM") as psum,
        ):
            identf = const.tile([P, P], F32)
            make_identity(nc, identf)
            ident = const.tile([P, P], F32R)
            nc.vector.tensor_copy(ident, identf)
            wt_sb = const.tile([P, DCH, K], F32R)
            nc.sync.dma_start(wt_sb, wt[:, :].rearrange("(a p) k -> p a k", p=P))
            cent_sb = const.tile([K, D], F32)
            nc.sync.dma_start(cent_sb, cent[:, :])
            ln8 = const.tile([K, 1], F32)
            nc.vector.memset(ln8, LN_EIGHTH)

            # Immortal PSUM tensors, manually double-buffered by bank-aligned
            # regions: a PE write to a bank is fatal while another engine reads
            # the same bank, so regions never straddle banks and cross-region
            # pairs always live in different banks (or have real WAR deps).
            # Banks: xtp 2 + lg 4 + vl 1 + asum 1 = 8.
            xtp = psum.tile([P, 2, D], F32R)         # transposed-x chunk, j%2
            lg = psum.tile([P, 2048], F32)          # logits, 1024*(b%2)+64j
            vl = psum.tile([K, D], F32)             # vlad (WAR-served, DVE-side)
            asum = psum.tile([K, 2], F32)           # a-sums (WAR-served; 2 cols for fp32r evenness)

            # PE pre-observes the gpsimd-produced identity so the first real
            # transpose carries a single sync wait (S3_LW allows only one).
            # Output goes to spare columns of the logits bank.
            nc.tensor.transpose(lg[:, 896:1024].bitcast(F32R), ident, ident)

            state = {}

            def phase1(b):
                r = b % 2
                lgb = 1024 * r
                xb = xnat_pool.tile([P, DCH, N], F32R, tag="xb")
                nc.sync.dma_start(
                    xb, x[b, :, :].rearrange("(a p) n -> p a n", p=P)
                )

                xt = xt_pool.tile([P, NJ, D], F32R, tag="xt")
                ss = smalls.tile([P, NJ], F32, tag="ss")
                nc.vector.memset(ss, 1.0)
                den = smalls.tile([P, NJ], F32, tag="den")
                nc.vector.memset(den, 1.0)
                lgc = sm_pool.tile([P, NJ, K], F32, tag="lgc")
                expt = sm_pool.tile([P, NJ, K], F32, tag="expt")
                atp = sm_pool.tile([P, NJ, K], F32R, tag="atp")

                # tiny matmul whose only dependency is the xb DMA: makes PE
                # observe that semaphore before the first real transpose.
                # pin the warm matmul after batch b-2's last PE instruction
                # (same-engine ordering, no extra sem) so it cannot be hoisted
                # to a point where exp(b-2) still reads this logits bank.
                warm = nc.tensor.matmul(
                    lg[0:2, lgb + 1020 : lgb + 1022],
                    xb[:, 0, 0:2],
                    xb[:, 0, 0:2],
                    start=True,
                    stop=True,
                    skip_group_check=True,
                )
                if "last_pe" in state:
                    add_dep_helper(
                        warm.ins,
                        state["last_pe"].ins,
                        sync=False,
                        reason="pin warm after prior phase2 PE work",
                    )

                for j, (n0, nj) in enumerate(NCHUNKS):
                    xr = j % 2
                    for a in range(DCH):
                        t_ins = nc.tensor.transpose(
                            xtp[:nj, xr, a * P : (a + 1) * P],
                            xb[:, a, n0 : n0 + nj],
                            ident,
                        )
                        if j == 0 and a == 0:
                            # force warm-matmul to schedule first so the xb DMA
                            # wait lands on it, not on this transpose
                            add_dep_helper(
                                t_ins.ins,
                                warm.ins,
                                sync=False,
                                reason="split DMA wait off first transpose",
                            )
                        nc.tensor.matmul(
                            lg[:nj, lgb + j * K : lgb + (j + 1) * K],
                            xb[:, a, n0 : n0 + nj],
                            wt_sb[:, a, :],
                            start=(a == 0),
                            stop=(a == DCH - 1),
                            skip_group_check=True,
                        )
                    # PSUM -> SBUF drain + sum(x^2), on opposite engines so the
                    # per-chunk pipeline is gated by neither alone
                    sq = scratch.tile([P, D], F32, tag="sq")
                    if j % 2 == 0:
                        # whole chunk on ScalarE: the copy is never queued
                        # behind another chunk's cross-engine work
                        nc.scalar.copy(xt[:nj, j], xtp[:nj, xr].bitcast(F32))
                        nc.scalar.copy(
                            lgc[:nj, j], lg[:nj, lgb + j * K : lgb + (j + 1) * K]
                        )
                        nc.scalar.activation(
                            sq[:nj],
                            xt[:nj, j].bitcast(F32),
                            ACTF.Square,
                            accum_out=ss[:nj, j : j + 1],
                        )
                    else:
                        # whole chunk on VectorE
                        # (tensor_tensor_reduce crashes the exec unit on this
                        # HW/compiler combo; scalar_tensor_tensor's fused accum
                        # computes the same sum of squares)
                        nc.vector.tensor_copy(xt[:nj, j], xtp[:nj, xr].bitcast(F32))
                        nc.vector.tensor_copy(
                            lgc[:nj, j], lg[:nj, lgb + j * K : lgb + (j + 1) * K]
                        )
                        nc.vector.scalar_tensor_tensor(
                            out=sq[:nj],
                            in0=xt[:nj, j].bitcast(F32),
                            scalar=1.0,
                            in1=xt[:nj, j].bitcast(F32),
                            op0=ALU.mult,
                            op1=ALU.mult,
                            accum_out=ss[:nj, j : j + 1],
                        )
                    # interleave the previous batch's aggregation matmuls into
                    # the back half of this batch's chunk loop: PE gets work
                    # while this batch's softmax tail runs on ACT/DVE
                    if b > 0 and j >= 5:
                        agg_chunks(b - 1, [2 * (j - 5), 2 * (j - 5) + 1])

                # sinv = 1/sqrt(ss) = exp(-0.5*ln(ss)) ; s = ss * sinv
                lss = smalls.tile([P, NJ], F32, tag="lss")
                nc.scalar.activation(lss, ss, ACTF.Ln)
                sinv = smalls.tile([P, NJ], F32, tag="sinv")
                nc.scalar.activation(sinv, lss, ACTF.Exp, scale=-0.5)
                s = smalls.tile([P, NJ], F32R, tag="s")
                nc.vector.tensor_tensor(s, ss, sinv, ALU.mult)

                # exp(logits * sinv); denominators reduced on VectorE
                for j, (n0, nj) in enumerate(NCHUNKS):
                    nc.scalar.activation(
                        expt[:nj, j],
                        lgc[:nj, j],
                        ACTF.Exp,
                        scale=sinv[:nj, j : j + 1],
                    )
                for j, (n0, nj) in enumerate(NCHUNKS):
                    nc.vector.tensor_reduce(
                        den[:nj, j : j + 1],
                        expt[:nj, j],
                        axis=mybir.AxisListType.X,
                        op=ALU.add,
                    )
                rden = smalls.tile([P, NJ], F32, tag="rden")
                nc.vector.reciprocal(rden, den)
                comb = smalls.tile([P, NJ], F32, tag="comb")
                nc.vector.tensor_tensor(comb, rden, sinv, ALU.mult)
                # a'[n,k] = exp * (1/den) * (1/s)
                for j, (n0, nj) in enumerate(NCHUNKS):
                    nc.vector.tensor_scalar_mul(
                        atp[:nj, j], expt[:nj, j], comb[:nj, j : j + 1]
                    )

                state[b] = (xt, atp, s)

            def agg_chunks(b, js):
                xt, atp, s = state[b]
                for j in js:
                    n0, nj = NCHUNKS[j]
                    nc.tensor.matmul(
                        vl,
                        atp[:nj, j],
                        xt[:nj, j],
                        start=(j == 0),
                        stop=(j == NJ - 1),
                    )
                    asum_mm = nc.tensor.matmul(
                        asum,
                        atp[:nj, j],
                        s[:nj, j : j + 1].to_broadcast((nj, 2)),
                        start=(j == 0),
                        stop=(j == NJ - 1),
                    )
                    if j == NJ - 1:
                        state["last_pe"] = asum_mm

            def phase2(b):
                state.pop(b)
                # negd = asum*c - vlad   (negated; sign fixed by the -1 below)
                negd = ep_pool.tile([K, D], F32, tag="negd")
                nc.vector.scalar_tensor_tensor(
                    out=negd,
                    in0=cent_sb,
                    scalar=asum[:, 0:1],
                    in1=vl[:, :],
                    op0=ALU.mult,
                    op1=ALU.subtract,
                )
                sq2 = ep_pool.tile([K, D], F32, tag="sq2")
                ssk = ep_pool.tile([K, 1], F32, tag="ssk")
                nc.vector.scalar_tensor_tensor(
                    out=sq2,
                    in0=negd,
                    scalar=1.0,
                    in1=negd,
                    op0=ALU.mult,
                    op1=ALU.mult,
                    accum_out=ssk,
                )
                # gk = (1/8) * rsqrt(ssk) == exp(-0.5*ln(ssk) + ln(1/8))
                lssk = ep_pool.tile([K, 1], F32, tag="lssk")
                nc.scalar.activation(lssk, ssk, ACTF.Ln)
                gk = ep_pool.tile([K, 1], F32, tag="gk")
                nc.scalar.activation(gk, lssk, ACTF.Exp, scale=-0.5, bias=ln8)
                ot = ep_pool.tile([K, D], F32, tag="ot")
                nc.vector.tensor_scalar(
                    out=ot,
                    in0=negd,
                    scalar1=gk,
                    scalar2=-1.0,
                    op0=ALU.mult,
                    op1=ALU.mult,
                )
                nc.gpsimd.dma_start(out[b, :, :], ot)

            for b in range(BPC):
                phase1(b)
                if b > 0:
                    phase2(b - 1)
            agg_chunks(BPC - 1, list(range(NJ)))
            phase2(BPC - 1)

    return nc


_NC = None


def _patch_act_tables():
    """Force every ScalarE activation onto the one table set that contains
    {copy, square, ln, exp} so the kernel pays a single ACT_TABLE_LOAD
    instead of thrashing between exp_and_others and natural_log."""
    import concourse.bacc as _bacc_mod
    orig = _bacc_mod.get_activation_tables

    def patched(arch):
        tables = dict(orig(arch))
        assert "natural_log_exp_and_others" in tables
        return {
            name: (funcs if name == "natural_log_exp_and_others" else set())
            for name, funcs in tables.items()
        }

    _bacc_mod.get_activation_tables = patched


def _get_nc():
    global _NC
    if _NC is None:
        _patch_act_tables()
        nc = bacc.Bacc("TRN2", target_bir_lowering=False)
        _emit(nc)
        nc.compile()
        _NC = nc
    return _NC


def _make_in_maps(x, conv_w, centroids):
    B = x.shape[0]
    xs = np.ascontiguousarray(x, dtype=np.float32).reshape(B, D, N)
    wt = np.ascontiguousarray(conv_w.T, dtype=np.float32)
    cent = np.ascontiguousarray(centroids, dtype=np.float32)
    in_maps = []
    for c in range(8):
        in_maps.append(
            {
                "x": np.ascontiguousarray(xs[c * BPC : (c + 1) * BPC]),
                "wt": wt,
                "cent": cent,
            }
        )
    return in_maps


def _run(x, conv_w, centroids, trace=False):
    nc = _get_nc()
    res = run_bass_kernel_spmd(
        nc,
        _make_in_maps(x, conv_w, centroids),
        core_ids=list(range(8)),
        trace=trace,
    )
    outs = [r["out"].reshape(BPC, K * D) for r in res.results]
    full = np.concatenate(outs, axis=0)
    return full, res


def kernel(x, conv_w, centroids):
    full, _ = _run(x, conv_w, centroids, trace=False)
    return full

